# revision 5
# baseline (speedup 1.0000x reference)
"""Trainium2 Bass kernel for nn_Grid1 (embedding_lookup / grid resample).

Math: the reference is torch-style grid_sample(bilinear, border, align_corners=True)
on a coordinate lattice that is an integer pixel lattice wrapped mod 1024:

    out[0, c, i, j] = grid[0, c, (ys + i) % 1024, (xs + j) % 1024]

(the normalized-coordinate round trip maps every sample to within 6.1e-5 of an
exact integer pixel, so bilinear weights degenerate to a pure gather; measured
L2 rel err of the pure gather vs the f32 reference is ~4e-5).

The 4096x4096 output is therefore a 4x4 periodic tiling of the (ys, xs)-rolled
1024x1024 grid. Sharding: each of the 8 cores owns one 128-row class of the
rolled grid (rows [128k, 128(k+1)) of the period), reads only its 4ch x 128 x 1024
band (2MB), and writes its 16 output blocks (4 vertical periods x 4 horizontal
periods worth, 32MB). The x-roll happens on-device via two segment DMAs per
block; HBM traffic per core = 2MB read + 32MB write (~memory roofline).
"""

from contextlib import ExitStack

import numpy as np

from concourse import bass, mybir
from concourse.bass_utils import run_bass_kernel_spmd

C = 4          # channels
G = 1024       # grid height/width (period)
HOUT = 4096    # output height/width
NCORES = 8
PB = G // NCORES      # rows of the period per core = 128 (= SBUF partitions)
V = HOUT // G         # vertical period repeats = 4
R = HOUT // G         # horizontal period repeats = 4

_NC_CACHE: dict = {}

# Set by test harnesses to capture an NTFF profile; harmless default.
TRACE = False
LAST_RESULT = None


def _build(xs: int) -> bass.Bass:
    """One SPMD program, specialized on the column shift xs.

    Raw bass (not Tile): the static-DMA lowering in this toolchain only
    supports a single sync-wait per DMA instruction, so sequencer-side
    wait_ge + per-channel load semaphores are used instead of Tile's
    auto-generated multi-sem waits.
    """
    nc = bass.Bass()
    g = nc.declare_dram_parameter("g", [C, PB, G], mybir.dt.float32, isOutput=False)
    o = nc.declare_dram_parameter("o", [C, V, PB, HOUT], mybir.dt.float32, isOutput=True)
    L = G - xs
    with ExitStack() as ctx:
        block = ctx.enter_context(nc.Block())
        ld_sems = [ctx.enter_context(nc.semaphore(f"ld{c}")) for c in range(C)]
        st_sem = ctx.enter_context(nc.semaphore("st"))
        t = ctx.enter_context(nc.sbuf_tensor("t", [PB, C * G], mybir.dt.float32))

        @block.sync
        def _(sync: bass.BassEngine):
            for c in range(C):
                sync.dma_start(t[:, c * G:(c + 1) * G], g[c]).then_inc(
                    ld_sems[c], 16)
            nstores = 0
            for c in range(C):
                sync.wait_ge(ld_sems[c], 16)
                for v in range(V):
                    # out[c, v, p, r*1024 + b] = t[p, c*1024 + (xs + b) % 1024]
                    dst = o[c, v].rearrange("p (r col) -> p r col", col=G)
                    srcA = t[:, c * G + xs:(c + 1) * G]
                    srcA = srcA.unsqueeze(1).broadcast_to((PB, R, L))
                    sync.dma_start(dst[:, :, 0:L], srcA).then_inc(st_sem, 16)
                    nstores += 1
                    if xs:
                        srcB = t[:, c * G:c * G + xs]
                        srcB = srcB.unsqueeze(1).broadcast_to((PB, R, xs))
                        sync.dma_start(dst[:, :, L:G], srcB).then_inc(st_sem, 16)
                        nstores += 1
            sync.wait_ge(st_sem, 16 * nstores)
    return nc


def _get_nc(xs: int) -> bass.Bass:
    if xs not in _NC_CACHE:
        _NC_CACHE[xs] = _build(xs)
    return _NC_CACHE[xs]


def kernel(grid, coordinate_start, h, w, support_resolution_h, support_resolution_w,
           **_unused):
    grid = np.asarray(grid, dtype=np.float32)
    cs = np.asarray(coordinate_start).astype(np.int64)
    xs = int(cs[0]) % G
    ys = int(cs[1]) % G
    assert grid.shape == (1, C, G, G), grid.shape
    assert int(h) == HOUT and int(w) == HOUT
    assert int(support_resolution_h) == G and int(support_resolution_w) == G

    g0 = grid[0]  # (C, G, G)
    in_maps = []
    for k in range(NCORES):
        rows = (ys + PB * k + np.arange(PB)) % G
        band = np.ascontiguousarray(g0[:, rows, :])  # (C, PB, G)
        in_maps.append({"g": band})

    nc = _get_nc(xs)
    res = run_bass_kernel_spmd(nc, in_maps, core_ids=list(range(NCORES)),
                               trace=TRACE)
    global LAST_RESULT
    LAST_RESULT = res

    full = np.empty((1, C, HOUT, HOUT), dtype=np.float32)
    for k in range(NCORES):
        r = np.asarray(res.results[k]["o"])  # (C, V, PB, HOUT)
        for v in range(V):
            base = v * G + PB * k
            full[0, :, base:base + PB, :] = r[:, v]
    return full
